# revision 4
# baseline (speedup 1.0000x reference)
"""Trainium2 Bass kernel for nn_C4ByteTransformer (4-step carry-propagation
softmax table lookup).

Contract: kernel(**inputs) takes FULL inputs (a_emb[4,256], b_emb[4,256],
W1[514,131072], W2_sum[131072,256], W2_carry[131072,2]) and returns the full
[4,256] float32 output.

Math: the tables are the canonical byte-add lookup structure (verified
exactly on host, with a numpy fallback otherwise):
  scores_i[k] = a_emb[i, a] + b_emb[i, b] + carry[c],  k = 512a + 2b + c
  weights = softmax(10*(scores - 2.5));  out_i = weights @ W2_sum;
  carry' = weights @ W2_carry,  W2_sum[k, (a+b+c) & 255] = 1,
  W2_carry[k, a+b+c >= 256] = 1.
Because exp is multiplicative over the separable score, with
EA[a] = exp(10 a_emb[i,a]), EB[b] = exp(10 b_emb[i,b]), F_c = exp(10 carry[c]):
  unnormalized w[k] = EA[a] EB[b] F_c,   Z = (sum EA)(sum EB)(F0 + F1)
  out_i[m] = (F0 cyc[m] + F1 cyc[(m-1) mod 256]) / Z
  carry'_1 = (F0 U + F1 (U + V)) / Z,   carry'_0 = 1 - carry'_1
where cyc = 256-point cyclic convolution of EA and EB,
U = sum_{a+b>=256} EA[a]EB[b], V = sum_{a+b=255} EA[a]EB[b].
The 131072-entry table never has to be touched: each step costs one
256-point convolution (four 128x128 Hankel-window matmuls built by
re-DMAing the step's EB through DRAM), plus suffix sums of EB via one
constant triangular matmul for U. The carry chain runs on ~30 scalars on
partition 0. Everything fits on ONE NeuronCore with ~600 KB of DMA
traffic total -- no collectives.
"""

import os

import numpy as np

NSTEP = 4
D = 256
NE = 131072

_CACHE = {}

LAST_EXEC_TIME_NS = None


def _build_nc():
    import concourse.bacc as bacc
    import concourse.bass as bass
    import concourse.mybir as mybir
    import concourse.tile as tile

    f32 = mybir.dt.float32
    mult = mybir.AluOpType.mult
    add = mybir.AluOpType.add
    subtract = mybir.AluOpType.subtract
    Exp = mybir.ActivationFunctionType.Exp
    AP = bass.AP

    nc = bacc.Bacc("TRN2", target_bir_lowering=False, debug=False,
                   num_devices=1)

    # Inputs (host pre-packed; see _prep_inputs).
    a8 = nc.dram_tensor("a8", [128, 2, NSTEP], f32, kind="ExternalInput")
    b8 = nc.dram_tensor("b8", [128, 2, NSTEP], f32, kind="ExternalInput")
    tri = nc.dram_tensor("tri", [128, 128], f32, kind="ExternalInput")
    onem = nc.dram_tensor("onem", [128, 128], f32, kind="ExternalInput")
    oh = nc.dram_tensor("oh", [8, NSTEP], f32, kind="ExternalInput")
    out = nc.dram_tensor("out", [NSTEP, D], f32, kind="ExternalOutput")

    # DRAM scratch for the data-dependent layout changes.
    ebd_d = nc.dram_tensor("ebd_d", [NSTEP, 384], f32)   # doubled EB rows
    cyc_d = nc.dram_tensor("cyc_d", [NSTEP, 260], f32)   # wrap + cyc per step
    l_d = nc.dram_tensor("l_d", [8, 1], f32)             # beta_i, alpha_i pairs

    with tile.TileContext(nc) as tc:
        with (
            tc.tile_pool(name="sb", bufs=1) as sb,
            tc.tile_pool(name="small", bufs=1) as small,
            tc.tile_pool(name="psA", bufs=1, space="PSUM") as psA,
            tc.tile_pool(name="psB", bufs=1, space="PSUM") as psB,
            tc.tile_pool(name="psC", bufs=1, space="PSUM") as psC,
            tc.tile_pool(name="psD", bufs=1, space="PSUM") as psD,
        ):
            bias0_128 = small.tile([128, 1], f32)
            nc.vector.memset(bias0_128[:], 0.0)
            bias0_1 = small.tile([1, 1], f32)
            nc.vector.memset(bias0_1[:], 0.0)
            one_1 = small.tile([1, 1], f32)
            nc.vector.memset(one_1[:], 1.0)

            # Resident inputs.
            a_sb = sb.tile([128, 2, NSTEP], f32)
            nc.sync.dma_start(a_sb[:], a8[:])
            b_sb = sb.tile([128, 2, NSTEP], f32)
            nc.sync.dma_start(b_sb[:], b8[:])
            tri_sb = sb.tile([128, 128], f32)
            nc.sync.dma_start(tri_sb[:], tri[:])
            one_sb = sb.tile([128, 128], f32)
            nc.sync.dma_start(one_sb[:], onem[:])
            oh_sb = sb.tile([8, NSTEP], f32)
            nc.sync.dma_start(oh_sb[:], oh[:])

            # EA_r[j, ah, i] = exp(10 a_emb[i, 128 ah + 127 - j]),
            # EB[p, bh, i] = exp(10 b_emb[i, 128 bh + p]).
            ea = sb.tile([128, 2, NSTEP], f32)
            nc.scalar.activation(ea[:], a_sb[:], Exp, bias=bias0_128[:],
                                 scale=10.0)
            eb = sb.tile([128, 2, NSTEP], f32)
            nc.scalar.activation(eb[:], b_sb[:], Exp, bias=bias0_128[:],
                                 scale=10.0)

            # ---- EBd rows: ebd_d[i, x] = EB_i[(x + 129) mod 256] ----
            nc.sync.dma_start(
                AP(ebd_d, 0, [[1, 127], [384, NSTEP]]), eb[1:128, 1:2, :]
            )
            nc.scalar.dma_start(
                AP(ebd_d, 127, [[1, 128], [384, NSTEP]]), eb[:, 0:1, :]
            )
            nc.scalar.dma_start(
                AP(ebd_d, 255, [[1, 128], [384, NSTEP]]), eb[:, 1:2, :]
            )

            # Hankel windows: wt[j, i, t, m] = ebd_d[i, 128 t + j + m].
            wt = sb.tile([128, NSTEP, 2, 128], f32)
            for t in range(2):
                nc.sync.dma_start(
                    wt[:, :, t, :],
                    AP(ebd_d, 128 * t, [[1, 128], [384, NSTEP], [1, 128]]),
                )

            # ---- Suffix sums: suf[p, tc, i] = sum_{b >= 128 tc + p + 1} EB_i[b]
            suf_ps = psA.tile([128, 2, NSTEP], f32)
            nc.tensor.matmul(suf_ps[:, 0, :], lhsT=tri_sb[:], rhs=eb[:, 0, :],
                             start=True, stop=False)
            nc.tensor.matmul(suf_ps[:, 0, :], lhsT=one_sb[:], rhs=eb[:, 1, :],
                             start=False, stop=True)
            nc.tensor.matmul(suf_ps[:, 1, :], lhsT=tri_sb[:], rhs=eb[:, 1, :],
                             start=True, stop=True)
            suf_sb = sb.tile([128, 2, NSTEP], f32)
            nc.vector.tensor_copy(out=suf_sb[:], in_=suf_ps[:])

            # ---- U/V element products; partition-reduce via ones matmul ----
            # scr[p, g, i], g in (U0, U1, V0, V1):
            #   U_ah term: EA_r[:, ah] * suf[:, 1 - ah];  V_ah: EA_r[:, ah] * EB[:, 1 - ah]
            scr = sb.tile([128, 4, NSTEP], f32)
            nc.vector.tensor_tensor(out=scr[:, 0, :], in0=ea[:, 0, :],
                                    in1=suf_sb[:, 1, :], op=mult)
            nc.vector.tensor_tensor(out=scr[:, 1, :], in0=ea[:, 1, :],
                                    in1=suf_sb[:, 0, :], op=mult)
            nc.vector.tensor_tensor(out=scr[:, 2, :], in0=ea[:, 0, :],
                                    in1=eb[:, 1, :], op=mult)
            nc.vector.tensor_tensor(out=scr[:, 3, :], in0=ea[:, 1, :],
                                    in1=eb[:, 0, :], op=mult)

            # red[0, 0:32] = [scr(U0 U1 V0 V1) | EA(ah0 ah1) | EB(ah0 ah1)] sums
            red_ps = psC.tile([1, 8, NSTEP], f32)
            ones_col = one_sb[:, 0:1]
            nc.tensor.matmul(red_ps[:, 0:4, :].opt(), lhsT=ones_col,
                             rhs=scr[:].opt(), start=True, stop=True)
            nc.tensor.matmul(red_ps[:, 4:6, :].opt(), lhsT=ones_col,
                             rhs=ea[:].opt(), start=True, stop=True)
            nc.tensor.matmul(red_ps[:, 6:8, :].opt(), lhsT=ones_col,
                             rhs=eb[:].opt(), start=True, stop=True)
            red_sb = small.tile([1, 8, NSTEP], f32)
            nc.vector.tensor_copy(out=red_sb[:], in_=red_ps[:])

            # sums[0, k, i]: k = 0:U, 1:V, 2:ZA, 3:ZB  (fold the ah pairs)
            sums = small.tile([1, 4, NSTEP], f32)
            nc.vector.tensor_tensor(
                out=sums[0:1, 0:1, :], in0=red_sb[0:1, 0, :].unsqueeze(1),
                in1=red_sb[0:1, 1, :].unsqueeze(1), op=add)
            nc.vector.tensor_tensor(
                out=sums[0:1, 1:2, :], in0=red_sb[0:1, 2, :].unsqueeze(1),
                in1=red_sb[0:1, 3, :].unsqueeze(1), op=add)
            nc.vector.tensor_tensor(
                out=sums[0:1, 2:3, :], in0=red_sb[0:1, 4, :].unsqueeze(1),
                in1=red_sb[0:1, 5, :].unsqueeze(1), op=add)
            nc.vector.tensor_tensor(
                out=sums[0:1, 3:4, :], in0=red_sb[0:1, 6, :].unsqueeze(1),
                in1=red_sb[0:1, 7, :].unsqueeze(1), op=add)
            zab = small.tile([1, NSTEP], f32)
            nc.vector.tensor_tensor(out=zab[:], in0=sums[0:1, 2, :],
                                    in1=sums[0:1, 3, :], op=mult)
            upv = small.tile([1, NSTEP], f32)
            nc.vector.tensor_tensor(out=upv[:], in0=sums[0:1, 0, :],
                                    in1=sums[0:1, 1, :], op=add)

            # ---- Carry recurrence on partition 0 ----
            carry = small.tile([1, 2], f32)
            nc.vector.memset(carry[0:1, 0:1], 1.0)
            nc.vector.memset(carry[0:1, 1:2], 0.0)
            fs = small.tile([1, 2], f32)
            t0 = small.tile([1, 1], f32)
            t1 = small.tile([1, 1], f32)
            zin = small.tile([1, 1], f32)
            scal = small.tile([1, 8], f32)  # (beta_i, alpha_i) pairs
            for i in range(NSTEP):
                nc.scalar.activation(fs[:], carry[:], Exp, bias=bias0_1[:],
                                     scale=10.0)
                nc.vector.tensor_tensor(out=t0[:], in0=fs[0:1, 0:1],
                                        in1=fs[0:1, 1:2], op=add)
                nc.vector.tensor_tensor(out=t1[:], in0=t0[:],
                                        in1=zab[0:1, i:i + 1], op=mult)
                nc.vector.reciprocal(zin[:], t1[:])
                # beta = F1/Z (pairs with cyc[m-1]), alpha = F0/Z
                nc.vector.tensor_tensor(out=scal[0:1, 2 * i:2 * i + 1],
                                        in0=fs[0:1, 1:2], in1=zin[:], op=mult)
                nc.vector.tensor_tensor(out=scal[0:1, 2 * i + 1:2 * i + 2],
                                        in0=fs[0:1, 0:1], in1=zin[:], op=mult)
                if i + 1 < NSTEP:
                    # c1' = alpha U + beta (U + V)
                    nc.vector.tensor_tensor(
                        out=t0[:], in0=scal[0:1, 2 * i + 1:2 * i + 2],
                        in1=sums[0:1, 0, i:i + 1], op=mult)
                    nc.vector.scalar_tensor_tensor(
                        out=carry[0:1, 1:2], in0=upv[0:1, i:i + 1],
                        scalar=scal[0:1, 2 * i:2 * i + 1], in1=t0[:],
                        op0=mult, op1=add)
                    nc.vector.tensor_tensor(out=carry[0:1, 0:1], in0=one_1[:],
                                            in1=carry[0:1, 1:2], op=subtract)

            # Scatter the 8 recurrence scalars onto partitions; L = oh * scal8.
            nc.sync.dma_start(AP(l_d, 0, [[1, 8]]), scal[:])
            l8 = small.tile([8, 1], f32)
            nc.sync.dma_start(l8[:], l_d[:])
            lmat = small.tile([8, NSTEP], f32)
            nc.vector.tensor_scalar(out=lmat[:], in0=oh_sb[:],
                                    scalar1=l8[:, 0:1], scalar2=None, op0=mult)

            # ---- Cyclic convolution: cyc[128 c + p] accumulated over ah ----
            cyc_ps = psB.tile([128, 2, NSTEP], f32)
            for i in range(NSTEP):
                for c in range(2):
                    for ah in range(2):
                        nc.tensor.matmul(
                            cyc_ps[:, c, i:i + 1],
                            lhsT=wt[:, i, (c - ah) % 2, :],
                            rhs=ea[:, ah, i:i + 1],
                            start=(ah == 0),
                            stop=(ah == 1),
                        )
            cyc_sb = sb.tile([128, 2, NSTEP], f32)
            nc.vector.tensor_copy(out=cyc_sb[:], in_=cyc_ps[:])

            # cyc_d[i, 0] = cyc[255] (wrap), cyc_d[i, 1 + m] = cyc[m]
            nc.sync.dma_start(
                AP(cyc_d, 1, [[1, 128], [260, NSTEP]]), cyc_sb[:, 0:1, :]
            )
            nc.sync.dma_start(
                AP(cyc_d, 129, [[1, 128], [260, NSTEP]]), cyc_sb[:, 1:2, :]
            )
            nc.scalar.dma_start(
                AP(cyc_d, 0, [[260, NSTEP]]), cyc_sb[127:128, 1:2, :]
            )
            # cyc8[2i + j, m] = cyc_d[i, j + m]  (j=0: shifted, j=1: aligned)
            cyc8 = sb.tile([8, 256], f32)
            nc.sync.dma_start(
                cyc8[:], AP(cyc_d, 0, [[260, NSTEP], [1, 2], [1, 256]])
            )

            # ---- Final combine: out[i, m] = beta_i cyc[m-1] + alpha_i cyc[m]
            out_ps = psD.tile([NSTEP, D], f32)
            nc.tensor.matmul(out_ps[:], lhsT=lmat[:], rhs=cyc8[:],
                             start=True, stop=True)
            out_sb = small.tile([NSTEP, D], f32)
            nc.vector.tensor_copy(out=out_sb[:], in_=out_ps[:])
            nc.sync.dma_start(out[:], out_sb[:])

    nc.compile()
    return nc


def _structure_ok(W1, W2_sum, W2_carry):
    """Exact check that the tables are the canonical byte-add structure."""
    k = np.arange(NE)
    a = k >> 9
    b = (k >> 1) & 255
    c = k & 1
    total = a + b + c
    if W1.shape != (514, NE) or W2_sum.shape != (NE, D):
        return False
    if W2_carry.shape != (NE, 2):
        return False
    if not (W1[a, k] == 1.0).all():
        return False
    if not (W1[256 + b, k] == 1.0).all():
        return False
    if not (W1[512 + c, k] == 1.0).all():
        return False
    if np.abs(W1).sum(dtype=np.float64) != 3.0 * NE:
        return False
    if not (W2_sum[k, total & 255] == 1.0).all():
        return False
    if np.abs(W2_sum).sum(dtype=np.float64) != float(NE):
        return False
    if not (W2_carry[k, (total >= 256).astype(np.int64)] == 1.0).all():
        return False
    if np.abs(W2_carry).sum(dtype=np.float64) != float(NE):
        return False
    return True


def _numpy_fallback(a_emb, b_emb, W1, W2_sum, W2_carry):
    carry = np.zeros(2, dtype=np.float64)
    carry[0] = 1.0
    outs = []
    W1 = W1.astype(np.float64)
    for i in range(NSTEP):
        x = np.concatenate([a_emb[i], b_emb[i], carry]).astype(np.float64)
        scores = x @ W1
        z = (scores - 2.5) * 10.0
        z -= z.max()
        w = np.exp(z)
        w /= w.sum()
        outs.append(w @ W2_sum.astype(np.float64))
        carry = w @ W2_carry.astype(np.float64)
    return np.stack(outs).astype(np.float32)


def _prep_inputs(a_emb, b_emb):
    p = np.arange(128)
    # a8[p, ah, i] = a_emb[i, 128 ah + 127 - p]
    a_r = a_emb[:, ::-1]  # a_r[i, y] = a_emb[i, 255 - y]
    a8 = np.ascontiguousarray(
        a_r.reshape(NSTEP, 2, 128)[:, ::-1, :].transpose(2, 1, 0)
    ).astype(np.float32)
    # b8[p, bh, i] = b_emb[i, 128 bh + p]
    b8 = np.ascontiguousarray(
        b_emb.reshape(NSTEP, 2, 128).transpose(2, 1, 0)
    ).astype(np.float32)
    tri = (p[:, None] >= p[None, :] + 1).astype(np.float32)
    onem = np.ones((128, 128), dtype=np.float32)
    oh = (np.arange(8)[:, None] // 2 == np.arange(NSTEP)[None, :]).astype(
        np.float32
    )
    return {"a8": a8, "b8": b8, "tri": tri, "onem": onem, "oh": oh}


def kernel(a_emb, b_emb, W1, W2_sum, W2_carry):
    global LAST_EXEC_TIME_NS
    a_emb = np.asarray(a_emb, dtype=np.float32)
    b_emb = np.asarray(b_emb, dtype=np.float32)
    W1 = np.asarray(W1, dtype=np.float32)
    W2_sum = np.asarray(W2_sum, dtype=np.float32)
    W2_carry = np.asarray(W2_carry, dtype=np.float32)

    if not _structure_ok(W1, W2_sum, W2_carry):
        return _numpy_fallback(a_emb, b_emb, W1, W2_sum, W2_carry)

    from concourse.bass_utils import run_bass_kernel_spmd

    if "nc" not in _CACHE:
        _CACHE["nc"] = _build_nc()
    nc = _CACHE["nc"]

    in_map = _prep_inputs(a_emb, b_emb)
    trace = os.environ.get("KERNEL_TRACE", "") == "1"
    res = run_bass_kernel_spmd(nc, [in_map], [0], trace=trace)
    LAST_EXEC_TIME_NS = res.exec_time_ns
    return np.asarray(res.results[0]["out"], dtype=np.float32)


# revision 9
# speedup vs baseline: 2.0886x; 2.0886x over previous
"""Trainium2 Bass kernel for nn_C4ByteTransformer (4-step carry-propagation
softmax table lookup).

Contract: kernel(**inputs) takes FULL inputs (a_emb[4,256], b_emb[4,256],
W1[514,131072], W2_sum[131072,256], W2_carry[131072,2]) and returns the full
[4,256] float32 output.

Math: the tables are the canonical byte-add lookup structure (verified
exactly on host, with a numpy fallback otherwise):
  scores_i[k] = a_emb[i, a] + b_emb[i, b] + carry[c],  k = 512a + 2b + c
  weights = softmax(10*(scores - 2.5));  out_i = weights @ W2_sum;
  carry' = weights @ W2_carry,  W2_sum[k, (a+b+c) & 255] = 1,
  W2_carry[k, a+b+c >= 256] = 1.
Because exp is multiplicative over the separable score, with
EA[a] = exp(10 a_emb[i,a]), EB[b] = exp(10 b_emb[i,b]), r = F0/F1
(= exp(10 - 20 carry_1)):
  out_i[m] = (r cyc[m] + cyc[(m-1) mod 256]) / (ZA ZB (1 + r))
  carry'_1 = (r U + U + V) / (ZA ZB (1 + r))
where cyc = 256-point cyclic convolution of EA and EB,
U = sum_{a+b>=256} EA[a]EB[b], V = sum_{a+b=255} EA[a]EB[b].
The 131072-entry table never has to be touched. Each step's convolution
is two accumulating matmuls whose lhsT is a step-masked exp(a) block
(off-step columns are exp(-50) ~ 0, so all eight matmuls accumulate into
one [4, 256] PSUM tile, landing step-on-partition with no transpose) and
whose rhs is a [128, 256] Hankel window of exp(b) re-DMAed through DRAM.
U comes from suffix sums of EB via one constant triangular matmul. The
carry chain runs on ~6 scalar ops per step on partition 0. Everything
fits on ONE NeuronCore with ~900 KB of DMA traffic total, no collectives.
"""

import os

import numpy as np

NSTEP = 4
D = 256
NE = 131072

_CACHE = {}

LAST_EXEC_TIME_NS = None


def _build_nc():
    import concourse.bacc as bacc
    import concourse.bass as bass
    import concourse.mybir as mybir
    import concourse.tile as tile

    f32 = mybir.dt.float32
    mult = mybir.AluOpType.mult
    add = mybir.AluOpType.add
    Exp = mybir.ActivationFunctionType.Exp
    AP = bass.AP

    nc = bacc.Bacc("TRN2", target_bir_lowering=False, debug=False,
                   num_devices=1)

    # Inputs (host pre-packed; see _prep_inputs).
    a8 = nc.dram_tensor("a8", [128, 2, NSTEP], f32, kind="ExternalInput")
    a8m = nc.dram_tensor("a8m", [128, 2, NSTEP, NSTEP], f32,
                         kind="ExternalInput")
    b8 = nc.dram_tensor("b8", [128, 2, NSTEP], f32, kind="ExternalInput")
    bT = nc.dram_tensor("bT", [NSTEP, 256], f32, kind="ExternalInput")
    tri = nc.dram_tensor("tri", [128, 128], f32, kind="ExternalInput")
    onem = nc.dram_tensor("onem", [128, 128], f32, kind="ExternalInput")
    out = nc.dram_tensor("out", [NSTEP, D], f32, kind="ExternalOutput")

    # DRAM scratch for the data-dependent layout changes.
    ebd_d = nc.dram_tensor("ebd_d", [NSTEP, 384], f32)  # doubled EB rows
    l_d = nc.dram_tensor("l_d", [NSTEP, 2], f32)        # (beta_i, alpha_i)

    with tile.TileContext(nc) as tc:
        with (
            tc.tile_pool(name="sb", bufs=1) as sb,
            tc.tile_pool(name="small", bufs=1) as small,
            tc.tile_pool(name="psA", bufs=1, space="PSUM") as psA,
            tc.tile_pool(name="psC", bufs=1, space="PSUM") as psC,
            tc.tile_pool(name="psD", bufs=1, space="PSUM") as psD,
        ):
            bias0_128 = small.tile([128, 1], f32)
            nc.vector.memset(bias0_128[:], 0.0)
            bias0_4 = small.tile([NSTEP, 1], f32)
            nc.vector.memset(bias0_4[:], 0.0)
            bias10_1 = small.tile([1, 1], f32)
            nc.vector.memset(bias10_1[:], 10.0)

            # Inputs, spread across queues; bT first (critical path).
            bT_sb = sb.tile([NSTEP, 256], f32)
            nc.sync.dma_start(bT_sb[:], bT[:])
            b8_sb = sb.tile([128, 2, NSTEP], f32)
            nc.scalar.dma_start(b8_sb[:], b8[:])
            a8m_sb = sb.tile([128, 2, NSTEP, NSTEP], f32)
            nc.gpsimd.dma_start(a8m_sb[:], a8m[:])
            a8_sb = sb.tile([128, 2, NSTEP], f32)
            nc.gpsimd.dma_start(a8_sb[:], a8[:])
            tri_sb = sb.tile([128, 128], f32)
            nc.scalar.dma_start(tri_sb[:], tri[:])
            one_sb = sb.tile([128, 128], f32)
            nc.sync.dma_start(one_sb[:], onem[:])

            # ebT[i, b] = exp(10 b_emb[i, b]) -- row-major for fast EBd writes.
            ebT = sb.tile([NSTEP, 256], f32)
            nc.scalar.activation(ebT[:], bT_sb[:], Exp, bias=bias0_4[:],
                                 scale=10.0)
            # ebd_d[i, x] = EB_i[(x + 129) mod 256], x in [0, 383)
            nc.sync.dma_start(
                AP(ebd_d, 0, [[384, NSTEP], [1, 127]]), ebT[:, 129:256]
            )
            nc.sync.dma_start(
                AP(ebd_d, 127, [[384, NSTEP], [1, 256]]), ebT[:]
            )

            # Hankel windows: wt2[j, i, t, m] = ebd_d[i, 128 (t%2) + j + m].
            # Slot t=2 duplicates t=0 so rhs [V1|V0] is a contiguous view.
            wt2 = sb.tile([128, NSTEP, 3, 128], f32)
            dma_engines = [nc.sync, nc.scalar, nc.gpsimd]
            for i in range(NSTEP):
                eng = dma_engines[i % 3]
                eng.dma_start(
                    wt2[:, i, 0:2, :],
                    AP(ebd_d, 384 * i, [[1, 128], [128, 2], [1, 128]]),
                )
                eng.dma_start(
                    wt2[:, i, 2, :],
                    AP(ebd_d, 384 * i, [[1, 128], [1, 128]]),
                )

            # exp of the a-side packings.
            ea = sb.tile([128, 2, NSTEP], f32)
            nc.scalar.activation(ea[:], a8_sb[:], Exp, bias=bias0_128[:],
                                 scale=10.0)
            eam = sb.tile([128, 2, NSTEP, NSTEP], f32)
            nc.scalar.activation(eam[:], a8m_sb[:], Exp, bias=bias0_128[:],
                                 scale=10.0)

            # ---- Suffix sums: suf[p, tc, i] = sum_{b >= 128 tc + p + 1} EB_i[b]
            suf_ps = psA.tile([128, 2, NSTEP], f32)
            eb = sb.tile([128, 2, NSTEP], f32)
            nc.scalar.activation(eb[:], b8_sb[:], Exp, bias=bias0_128[:],
                                 scale=10.0)
            nc.tensor.matmul(suf_ps[:, 0, :], lhsT=tri_sb[:], rhs=eb[:, 0, :],
                             start=True, stop=False)
            nc.tensor.matmul(suf_ps[:, 0, :], lhsT=one_sb[:], rhs=eb[:, 1, :],
                             start=False, stop=True)
            nc.tensor.matmul(suf_ps[:, 1, :], lhsT=tri_sb[:], rhs=eb[:, 1, :],
                             start=True, stop=True)
            suf_sb = sb.tile([128, 2, NSTEP], f32)
            nc.vector.tensor_copy(out=suf_sb[:], in_=suf_ps[:])

            # ---- U/V element products; partition-reduce via ones matmul ----
            scr = sb.tile([128, 4, NSTEP], f32)
            nc.vector.tensor_tensor(out=scr[:, 0, :], in0=ea[:, 0, :],
                                    in1=suf_sb[:, 1, :], op=mult)
            nc.vector.tensor_tensor(out=scr[:, 1, :], in0=ea[:, 1, :],
                                    in1=suf_sb[:, 0, :], op=mult)
            nc.vector.tensor_tensor(out=scr[:, 2, :], in0=ea[:, 0, :],
                                    in1=eb[:, 1, :], op=mult)
            nc.vector.tensor_tensor(out=scr[:, 3, :], in0=ea[:, 1, :],
                                    in1=eb[:, 0, :], op=mult)

            red_ps = psC.tile([1, 8, NSTEP], f32)
            ones_col = one_sb[:, 0:1]
            nc.tensor.matmul(red_ps[:, 0:4, :].opt(), lhsT=ones_col,
                             rhs=scr[:].opt(), start=True, stop=True)
            nc.tensor.matmul(red_ps[:, 4:6, :].opt(), lhsT=ones_col,
                             rhs=ea[:].opt(), start=True, stop=True)
            nc.tensor.matmul(red_ps[:, 6:8, :].opt(), lhsT=ones_col,
                             rhs=eb[:].opt(), start=True, stop=True)
            red_sb = small.tile([1, 8, NSTEP], f32)
            nc.vector.tensor_copy(out=red_sb[:], in_=red_ps[:])

            # sums[0, k, i]: k = 0:U, 1:V, 2:ZA, 3:ZB (fold the ah pairs)
            sums = small.tile([1, 4, NSTEP], f32)
            for k in range(4):
                nc.vector.tensor_tensor(
                    out=sums[0:1, k:k + 1, :],
                    in0=red_sb[0:1, 2 * k, :].unsqueeze(1),
                    in1=red_sb[0:1, 2 * k + 1, :].unsqueeze(1), op=add)
            zab = small.tile([1, NSTEP], f32)
            nc.vector.tensor_tensor(out=zab[:], in0=sums[0:1, 2, :],
                                    in1=sums[0:1, 3, :], op=mult)
            upv = small.tile([1, NSTEP], f32)
            nc.vector.tensor_tensor(out=upv[:], in0=sums[0:1, 0, :],
                                    in1=sums[0:1, 1, :], op=add)

            # ---- Carry recurrence on partition 0 (r = F0/F1 form) ----
            cc = small.tile([1, 1], f32)
            nc.vector.memset(cc[:], 0.0)  # carry_1 = 0
            rr = small.tile([1, 1], f32)
            num = small.tile([1, 1], f32)
            den = small.tile([1, 1], f32)
            zden = small.tile([1, 1], f32)
            scal = small.tile([1, 8], f32)  # (beta_i, alpha_i) pairs
            for i in range(NSTEP):
                # r = exp(10 - 20 c1)
                nc.scalar.activation(rr[:], cc[:], Exp, bias=bias10_1[:],
                                     scale=-20.0)
                nc.vector.scalar_tensor_tensor(
                    out=den[:], in0=zab[0:1, i:i + 1], scalar=rr[:],
                    in1=zab[0:1, i:i + 1], op0=mult, op1=add)
                nc.vector.scalar_tensor_tensor(
                    out=num[:], in0=sums[0:1, 0, i:i + 1], scalar=rr[:],
                    in1=upv[0:1, i:i + 1], op0=mult, op1=add)
                nc.vector.reciprocal(zden[:], den[:])
                nc.vector.tensor_copy(out=scal[0:1, 2 * i:2 * i + 1],
                                      in_=zden[:])
                nc.vector.tensor_tensor(out=scal[0:1, 2 * i + 1:2 * i + 2],
                                        in0=rr[:], in1=zden[:], op=mult)
                if i + 1 < NSTEP:
                    nc.vector.tensor_tensor(out=cc[:], in0=num[:],
                                            in1=zden[:], op=mult)

            # Scatter (beta_i, alpha_i) onto partitions 0..3.
            nc.sync.dma_start(AP(l_d, 0, [[1, 8]]), scal[:])
            lsb = small.tile([NSTEP, 2], f32)
            nc.sync.dma_start(lsb[:], l_d[:])

            # ---- Convolutions: 8 matmuls accumulate into prt[i, m] ----
            # lhsT = masked exp(a) block (off-step columns ~ exp(-50));
            # rhs ah=0: [V0|V1], ah=1: [V1|V0].
            prt = psD.tile([NSTEP, 256], f32)
            for i in range(NSTEP):
                for ah in range(2):
                    nc.tensor.matmul(
                        prt[:],
                        lhsT=eam[:, ah, i, :],
                        rhs=wt2[:, i, ah:ah + 2, :].opt(),
                        start=(i == 0 and ah == 0),
                        stop=(i == NSTEP - 1 and ah == 1),
                    )

            # rt[i, 1 + m] = cyc_i[m]; rt[i, 0] = cyc_i[255]
            rt = small.tile([NSTEP, 258], f32)
            nc.vector.tensor_copy(out=rt[:, 1:257], in_=prt[:])
            nc.vector.tensor_copy(out=rt[:, 0:1], in_=rt[:, 256:257])

            # out[i, m] = alpha_i cyc[m] + beta_i cyc[m-1]
            comb = small.tile([NSTEP, D], f32)
            nc.vector.tensor_scalar(out=comb[:], in0=rt[:, 1:257],
                                    scalar1=lsb[:, 1:2], scalar2=None,
                                    op0=mult)
            nc.vector.scalar_tensor_tensor(out=comb[:], in0=rt[:, 0:256],
                                           scalar=lsb[:, 0:1], in1=comb[:],
                                           op0=mult, op1=add)
            nc.sync.dma_start(out[:], comb[:])

    nc.compile()
    return nc


def _structure_ok(W1, W2_sum, W2_carry):
    """Exact check that the tables are the canonical byte-add structure."""
    k = np.arange(NE)
    a = k >> 9
    b = (k >> 1) & 255
    c = k & 1
    total = a + b + c
    if W1.shape != (514, NE) or W2_sum.shape != (NE, D):
        return False
    if W2_carry.shape != (NE, 2):
        return False
    if not (W1[a, k] == 1.0).all():
        return False
    if not (W1[256 + b, k] == 1.0).all():
        return False
    if not (W1[512 + c, k] == 1.0).all():
        return False
    if np.abs(W1).sum(dtype=np.float64) != 3.0 * NE:
        return False
    if not (W2_sum[k, total & 255] == 1.0).all():
        return False
    if np.abs(W2_sum).sum(dtype=np.float64) != float(NE):
        return False
    if not (W2_carry[k, (total >= 256).astype(np.int64)] == 1.0).all():
        return False
    if np.abs(W2_carry).sum(dtype=np.float64) != float(NE):
        return False
    return True


def _numpy_fallback(a_emb, b_emb, W1, W2_sum, W2_carry):
    carry = np.zeros(2, dtype=np.float64)
    carry[0] = 1.0
    outs = []
    W1 = W1.astype(np.float64)
    for i in range(NSTEP):
        x = np.concatenate([a_emb[i], b_emb[i], carry]).astype(np.float64)
        scores = x @ W1
        z = (scores - 2.5) * 10.0
        z -= z.max()
        w = np.exp(z)
        w /= w.sum()
        outs.append(w @ W2_sum.astype(np.float64))
        carry = w @ W2_carry.astype(np.float64)
    return np.stack(outs).astype(np.float32)


def _prep_inputs(a_emb, b_emb):
    p = np.arange(128)
    # a8[p, ah, i] = a_emb[i, 128 ah + 127 - p]
    a_r = a_emb[:, ::-1]
    a8 = np.ascontiguousarray(
        a_r.reshape(NSTEP, 2, 128)[:, ::-1, :].transpose(2, 1, 0)
    ).astype(np.float32)
    # a8m: step-masked copy (off-step columns -5 -> exp(10x) ~ 2e-22)
    a8m = np.full((128, 2, NSTEP, NSTEP), -5.0, dtype=np.float32)
    for i in range(NSTEP):
        a8m[:, :, i, i] = a8[:, :, i]
    # b8[p, bh, i] = b_emb[i, 128 bh + p]
    b8 = np.ascontiguousarray(
        b_emb.reshape(NSTEP, 2, 128).transpose(2, 1, 0)
    ).astype(np.float32)
    bT = np.ascontiguousarray(b_emb).astype(np.float32)
    tri = (p[:, None] >= p[None, :] + 1).astype(np.float32)
    onem = np.ones((128, 128), dtype=np.float32)
    return {"a8": a8, "a8m": a8m, "b8": b8, "bT": bT, "tri": tri,
            "onem": onem}


def kernel(a_emb, b_emb, W1, W2_sum, W2_carry):
    global LAST_EXEC_TIME_NS
    a_emb = np.asarray(a_emb, dtype=np.float32)
    b_emb = np.asarray(b_emb, dtype=np.float32)
    W1 = np.asarray(W1, dtype=np.float32)
    W2_sum = np.asarray(W2_sum, dtype=np.float32)
    W2_carry = np.asarray(W2_carry, dtype=np.float32)

    if not _structure_ok(W1, W2_sum, W2_carry):
        return _numpy_fallback(a_emb, b_emb, W1, W2_sum, W2_carry)

    from concourse.bass_utils import run_bass_kernel_spmd

    if "nc" not in _CACHE:
        _CACHE["nc"] = _build_nc()
    nc = _CACHE["nc"]

    in_map = _prep_inputs(a_emb, b_emb)
    trace = os.environ.get("KERNEL_TRACE", "") == "1"
    res = run_bass_kernel_spmd(nc, [in_map], [0], trace=trace)
    LAST_EXEC_TIME_NS = res.exec_time_ns
    return np.asarray(res.results[0]["out"], dtype=np.float32)


# revision 18
# speedup vs baseline: 2.3554x; 1.1277x over previous
"""Trainium2 Bass kernel for nn_C4ByteTransformer (4-step carry-propagation
softmax table lookup).

Contract: kernel(**inputs) takes FULL inputs (a_emb[4,256], b_emb[4,256],
W1[514,131072], W2_sum[131072,256], W2_carry[131072,2]) and returns the full
[4,256] float32 output.

Math: the tables are the canonical byte-add lookup structure (verified
exactly on host, with a numpy fallback otherwise):
  scores_i[k] = a_emb[i, a] + b_emb[i, b] + carry[c],  k = 512a + 2b + c
  weights = softmax(10*(scores - 2.5));  out_i = weights @ W2_sum;
  carry' = weights @ W2_carry,  W2_sum[k, (a+b+c) & 255] = 1,
  W2_carry[k, a+b+c >= 256] = 1.
Because exp is multiplicative over the separable score, with
EA[a] = exp(10 a_emb[i,a]), EB[b] = exp(10 b_emb[i,b]), r = F0/F1
(= exp(10 - 20 carry_1)):
  out_i[m] = (r cyc[m] + cyc[(m-1) mod 256]) / (ZA ZB (1 + r))
  carry'_1 = (r U + U + V) / (ZA ZB (1 + r))
where cyc = 256-point cyclic convolution of EA and EB,
U = sum_{a+b>=256} EA[a]EB[b], V = sum_{a+b=255} EA[a]EB[b].
The 131072-entry table never has to be touched. Each step's convolution
is two accumulating matmuls whose lhsT is a step-masked exp(a) block
(off-step columns are exp(-50) ~ 0, so all eight matmuls accumulate into
one [4, 256] PSUM tile, landing step-on-partition with no transpose) and
whose rhs is a [128, 256] Hankel window of exp(b) re-DMAed through DRAM.
U comes from suffix sums of EB via one constant triangular matmul. The
carry chain runs on ~6 scalar ops per step on partition 0. Everything
fits on ONE NeuronCore with ~900 KB of DMA traffic total, no collectives.
"""

import os

import numpy as np

NSTEP = 4
D = 256
NE = 131072

_CACHE = {}

LAST_EXEC_TIME_NS = None


def _build_nc():
    import concourse.bacc as bacc
    import concourse.bass as bass
    import concourse.mybir as mybir
    import concourse.tile as tile

    f32 = mybir.dt.float32
    f32r = mybir.dt.float32r
    mult = mybir.AluOpType.mult
    add = mybir.AluOpType.add
    Exp = mybir.ActivationFunctionType.Exp
    AP = bass.AP

    nc = bacc.Bacc("TRN2", target_bir_lowering=False, debug=False,
                   num_devices=1)

    # Inputs (host pre-packed; see _prep_inputs).
    a8 = nc.dram_tensor("a8", [128, 2, NSTEP], f32, kind="ExternalInput")
    a8m = nc.dram_tensor("a8m", [128, 2, NSTEP, NSTEP], f32,
                         kind="ExternalInput")
    b8 = nc.dram_tensor("b8", [128, 2, NSTEP], f32, kind="ExternalInput")
    bT = nc.dram_tensor("bT", [NSTEP, 256], f32, kind="ExternalInput")
    tri = nc.dram_tensor("tri", [128, 128], f32, kind="ExternalInput")
    onem = nc.dram_tensor("onem", [128, 128], f32, kind="ExternalInput")
    out = nc.dram_tensor("out", [NSTEP, D], f32, kind="ExternalOutput")

    # DRAM scratch for the data-dependent layout change (EB -> Hankel rows).
    ebd_d = nc.dram_tensor("ebd_d", [NSTEP, 640], f32r)

    with tile.TileContext(nc) as tc:
        with (
            tc.tile_pool(name="sb", bufs=1) as sb,
            tc.tile_pool(name="small", bufs=1) as small,
            tc.tile_pool(name="psA", bufs=1, space="PSUM") as psA,
            tc.tile_pool(name="psC", bufs=1, space="PSUM") as psC,
            tc.tile_pool(name="psD", bufs=1, space="PSUM") as psD,
        ):
            bias0_128 = small.tile([128, 1], f32)
            nc.vector.memset(bias0_128[:], 0.0)
            bias10_1 = small.tile([1, 1], f32)
            nc.vector.memset(bias10_1[:], 10.0)

            # Inputs, spread across queues; bT first (critical path).
            bT_sb = sb.tile([NSTEP, 256], f32)
            nc.sync.dma_start(bT_sb[:], bT[:])
            b8_sb = sb.tile([128, 2, NSTEP], f32)
            nc.scalar.dma_start(b8_sb[:], b8[:])
            a8_sb = sb.tile([128, 2, NSTEP], f32)
            nc.gpsimd.dma_start(a8_sb[:], a8[:])
            a8m_sb = sb.tile([128, 2, NSTEP, NSTEP], f32)
            nc.gpsimd.dma_start(a8m_sb[:], a8m[:])
            tri_sb = sb.tile([128, 128], f32)
            nc.scalar.dma_start(tri_sb[:], tri[:])
            one_sb = sb.tile([128, 128], f32)
            nc.sync.dma_start(one_sb[:], onem[:])

            # ebT[i, b] = exp(10 b_emb[i, b]) -- row-major for fast EBd writes.
            ebT = sb.tile([NSTEP, 256], f32r)
            nc.scalar.activation(ebT[:], bT_sb[:], Exp,
                                 bias=bias0_128[0:NSTEP, :], scale=10.0)
            # ebd_d[i, x] = EB_i[(x + 129) mod 256], x in [0, 639)
            nc.sync.dma_start(
                AP(ebd_d, 0, [[640, NSTEP], [1, 127]]), ebT[:, 129:256]
            )
            nc.scalar.dma_start(
                AP(ebd_d, 127, [[640, NSTEP], [1, 256]]), ebT[:]
            )
            nc.gpsimd.dma_start(
                AP(ebd_d, 383, [[640, NSTEP], [1, 256]]), ebT[:]
            )

            # Hankel windows: wt[j, i, x] = ebd_d[i, j + x] = EB_i[(j + x +
            # 129) mod 256]; rhs views [V0|V1] = x 0:256, [V1|V0] = 128:384.
            wt = sb.tile([128, NSTEP, 512], f32r)
            dma_engines = [nc.sync, nc.scalar, nc.gpsimd, nc.sync]
            for i in range(NSTEP):
                dma_engines[i].dma_start(
                    wt[:, i, :], AP(ebd_d, 640 * i, [[1, 128], [1, 512]])
                )

            # exp of the b/a-side packings (eb first: it gates suf -> sums).
            eb = sb.tile([128, 2, NSTEP], f32)
            nc.scalar.activation(eb[:], b8_sb[:], Exp, bias=bias0_128[:],
                                 scale=10.0)
            ea = sb.tile([128, 2, NSTEP], f32)
            nc.scalar.activation(ea[:], a8_sb[:], Exp, bias=bias0_128[:],
                                 scale=10.0)
            eam = sb.tile([128, 2, NSTEP, NSTEP], f32r)
            nc.scalar.activation(eam[:], a8m_sb[:], Exp, bias=bias0_128[:],
                                 scale=10.0)

            # ---- Suffix sums: suf[p, tc, i] = sum_{b >= 128 tc + p + 1} EB_i[b]
            suf_ps = psA.tile([128, 2, NSTEP], f32)
            nc.tensor.matmul(suf_ps[:, 0, :], lhsT=tri_sb[:], rhs=eb[:, 0, :],
                             start=True, stop=False)
            nc.tensor.matmul(suf_ps[:, 0, :], lhsT=one_sb[:], rhs=eb[:, 1, :],
                             start=False, stop=True)
            nc.tensor.matmul(suf_ps[:, 1, :], lhsT=tri_sb[:], rhs=eb[:, 1, :],
                             start=True, stop=True)
            suf_sb = sb.tile([128, 2, NSTEP], f32)
            nc.vector.tensor_copy(out=suf_sb[:], in_=suf_ps[:])

            # ---- U/V element products; partition-reduce via ones matmul ----
            scr = sb.tile([128, 4, NSTEP], f32)
            nc.vector.tensor_tensor(out=scr[:, 0, :], in0=ea[:, 0, :],
                                    in1=suf_sb[:, 1, :], op=mult)
            nc.vector.tensor_tensor(out=scr[:, 1, :], in0=ea[:, 1, :],
                                    in1=suf_sb[:, 0, :], op=mult)
            nc.vector.tensor_tensor(out=scr[:, 2, :], in0=ea[:, 0, :],
                                    in1=eb[:, 1, :], op=mult)
            nc.vector.tensor_tensor(out=scr[:, 3, :], in0=ea[:, 1, :],
                                    in1=eb[:, 0, :], op=mult)

            red_ps = psC.tile([1, 8, NSTEP], f32)
            ones_col = one_sb[:, 0:1]
            nc.tensor.matmul(red_ps[:, 0:4, :].opt(), lhsT=ones_col,
                             rhs=scr[:].opt(), start=True, stop=True)
            nc.tensor.matmul(red_ps[:, 4:6, :].opt(), lhsT=ones_col,
                             rhs=ea[:].opt(), start=True, stop=True)
            nc.tensor.matmul(red_ps[:, 6:8, :].opt(), lhsT=ones_col,
                             rhs=eb[:].opt(), start=True, stop=True)
            red_sb = small.tile([1, 8, NSTEP], f32)
            nc.vector.tensor_copy(out=red_sb[:], in_=red_ps[:])

            # sums[0, k, i]: k = 0:U, 1:V, 2:ZA, 3:ZB (fold the ah pairs)
            sums = small.tile([1, 4, NSTEP], f32)
            for k in range(4):
                nc.vector.tensor_tensor(
                    out=sums[0:1, k:k + 1, :],
                    in0=red_sb[0:1, 2 * k, :].unsqueeze(1),
                    in1=red_sb[0:1, 2 * k + 1, :].unsqueeze(1), op=add)
            zab = small.tile([1, NSTEP], f32)
            nc.vector.tensor_tensor(out=zab[:], in0=sums[0:1, 2, :],
                                    in1=sums[0:1, 3, :], op=mult)
            upv = small.tile([1, NSTEP], f32)
            nc.vector.tensor_tensor(out=upv[:], in0=sums[0:1, 0, :],
                                    in1=sums[0:1, 1, :], op=add)

            # ---- Carry recurrence on partition 0 (r = F0/F1 form) ----
            cc = small.tile([1, 1], f32)
            nc.vector.memset(cc[:], 0.0)  # carry_1 = 0
            rr = small.tile([1, 1], f32)
            num = small.tile([1, 1], f32)
            den = small.tile([1, 1], f32)
            scal = small.tile([1, 8], f32)  # (beta_i, alpha_i) pairs
            for i in range(NSTEP):
                # r = exp(10 - 20 c1)
                nc.scalar.activation(rr[:], cc[:], Exp, bias=bias10_1[:],
                                     scale=-20.0)
                nc.vector.scalar_tensor_tensor(
                    out=den[:], in0=zab[0:1, i:i + 1], scalar=rr[:],
                    in1=zab[0:1, i:i + 1], op0=mult, op1=add)
                nc.vector.scalar_tensor_tensor(
                    out=num[:], in0=sums[0:1, 0, i:i + 1], scalar=rr[:],
                    in1=upv[0:1, i:i + 1], op0=mult, op1=add)
                zden = scal[0:1, 2 * i:2 * i + 1]  # beta_i = 1/den
                nc.vector.reciprocal(zden, den[:])
                nc.vector.tensor_tensor(out=scal[0:1, 2 * i + 1:2 * i + 2],
                                        in0=rr[:], in1=zden, op=mult)
                if i + 1 < NSTEP:
                    nc.vector.tensor_tensor(out=cc[:], in0=num[:],
                                            in1=zden, op=mult)

            # Scatter (beta_i, alpha_i) onto partitions 0..3 (SBUF -> SBUF).
            lsb = small.tile([NSTEP, 2], f32)
            nc.sync.dma_start(lsb[:], scal[:])

            # ---- Convolutions: 8 matmuls accumulate into prt[i, m] ----
            # lhsT = masked exp(a) block (off-step columns ~ exp(-50));
            # rhs ah=0: [V0|V1], ah=1: [V1|V0]. float32r single-pass PE mode.
            prt = psD.tile([NSTEP, 256], f32)
            for i in range(NSTEP):
                for ah in range(2):
                    nc.tensor.matmul(
                        prt[:],
                        lhsT=eam[:, ah, i, :],
                        rhs=wt[:, i, 128 * ah:128 * ah + 256],
                        start=(i == 0 and ah == 0),
                        stop=(i == NSTEP - 1 and ah == 1),
                    )

            # rt[i, 1 + m] = cyc_i[m]; rt[i, 0] = cyc_i[255]
            rt = small.tile([NSTEP, 258], f32)
            nc.vector.tensor_copy(out=rt[:, 1:257], in_=prt[:])
            nc.vector.tensor_copy(out=rt[:, 0:1], in_=rt[:, 256:257])

            # out[i, m] = alpha_i cyc[m] + beta_i cyc[m-1]
            comb = small.tile([NSTEP, D], f32)
            nc.vector.tensor_scalar(out=comb[:], in0=rt[:, 1:257],
                                    scalar1=lsb[:, 1:2], scalar2=None,
                                    op0=mult)
            nc.vector.scalar_tensor_tensor(out=comb[:], in0=rt[:, 0:256],
                                           scalar=lsb[:, 0:1], in1=comb[:],
                                           op0=mult, op1=add)
            nc.sync.dma_start(out[:], comb[:])

    nc.compile()
    return nc


def _structure_ok(W1, W2_sum, W2_carry):
    """Exact check that the tables are the canonical byte-add structure."""
    k = np.arange(NE)
    a = k >> 9
    b = (k >> 1) & 255
    c = k & 1
    total = a + b + c
    if W1.shape != (514, NE) or W2_sum.shape != (NE, D):
        return False
    if W2_carry.shape != (NE, 2):
        return False
    if not (W1[a, k] == 1.0).all():
        return False
    if not (W1[256 + b, k] == 1.0).all():
        return False
    if not (W1[512 + c, k] == 1.0).all():
        return False
    if np.abs(W1).sum(dtype=np.float64) != 3.0 * NE:
        return False
    if not (W2_sum[k, total & 255] == 1.0).all():
        return False
    if np.abs(W2_sum).sum(dtype=np.float64) != float(NE):
        return False
    if not (W2_carry[k, (total >= 256).astype(np.int64)] == 1.0).all():
        return False
    if np.abs(W2_carry).sum(dtype=np.float64) != float(NE):
        return False
    return True


def _numpy_fallback(a_emb, b_emb, W1, W2_sum, W2_carry):
    carry = np.zeros(2, dtype=np.float64)
    carry[0] = 1.0
    outs = []
    W1 = W1.astype(np.float64)
    for i in range(NSTEP):
        x = np.concatenate([a_emb[i], b_emb[i], carry]).astype(np.float64)
        scores = x @ W1
        z = (scores - 2.5) * 10.0
        z -= z.max()
        w = np.exp(z)
        w /= w.sum()
        outs.append(w @ W2_sum.astype(np.float64))
        carry = w @ W2_carry.astype(np.float64)
    return np.stack(outs).astype(np.float32)


def _prep_inputs(a_emb, b_emb):
    p = np.arange(128)
    # a8[p, ah, i] = a_emb[i, 128 ah + 127 - p]
    a_r = a_emb[:, ::-1]
    a8 = np.ascontiguousarray(
        a_r.reshape(NSTEP, 2, 128)[:, ::-1, :].transpose(2, 1, 0)
    ).astype(np.float32)
    # a8m: step-masked copy (off-step columns -5 -> exp(10x) ~ 2e-22)
    a8m = np.full((128, 2, NSTEP, NSTEP), -5.0, dtype=np.float32)
    for i in range(NSTEP):
        a8m[:, :, i, i] = a8[:, :, i]
    # b8[p, bh, i] = b_emb[i, 128 bh + p]
    b8 = np.ascontiguousarray(
        b_emb.reshape(NSTEP, 2, 128).transpose(2, 1, 0)
    ).astype(np.float32)
    bT = np.ascontiguousarray(b_emb).astype(np.float32)
    tri = (p[:, None] >= p[None, :] + 1).astype(np.float32)
    onem = np.ones((128, 128), dtype=np.float32)
    return {"a8": a8, "a8m": a8m, "b8": b8, "bT": bT, "tri": tri,
            "onem": onem}


def kernel(a_emb, b_emb, W1, W2_sum, W2_carry):
    global LAST_EXEC_TIME_NS
    a_emb = np.asarray(a_emb, dtype=np.float32)
    b_emb = np.asarray(b_emb, dtype=np.float32)
    W1 = np.asarray(W1, dtype=np.float32)
    W2_sum = np.asarray(W2_sum, dtype=np.float32)
    W2_carry = np.asarray(W2_carry, dtype=np.float32)

    if not _structure_ok(W1, W2_sum, W2_carry):
        return _numpy_fallback(a_emb, b_emb, W1, W2_sum, W2_carry)

    from concourse.bass_utils import run_bass_kernel_spmd

    if "nc" not in _CACHE:
        _CACHE["nc"] = _build_nc()
    nc = _CACHE["nc"]

    in_map = _prep_inputs(a_emb, b_emb)
    trace = os.environ.get("KERNEL_TRACE", "") == "1"
    res = run_bass_kernel_spmd(nc, [in_map], [0], trace=trace)
    LAST_EXEC_TIME_NS = res.exec_time_ns
    return np.asarray(res.results[0]["out"], dtype=np.float32)
